# revision 1
# baseline (speedup 1.0000x reference)
"""Two-level VQ codebook assignment on 8 Trainium2 NeuronCores.

Algorithm (matches reference):
  outer = argmin_k ||x[:, :16] - OC[k]||^2          (64 outer centers)
  inner = argmin_j ||x[:, 16:48] - IC[outer, j]||^2 (5 inner centers of winner)
  out   = outer * 5 + inner                          (int32)

Design (v2): avoid computing all 320 inner scores per point. Per 128-point
chunk:
  1. PE: transpose packed [xh1|xl1|xh1] -> xt, one K=51 matmul vs W1 [51,64]
     gives outer scores O = 2*x1.OC - ||OC||^2 in fp16 hi/lo precision.
  2. DVE (batched over 8 chunks): segmented reduce-max -> m*; H = (O == m*)
     one-hot over the 64 clusters (fp16, SBUF).
  3. PE: transpose H; matmul H @ Wf [64, 166] gathers the winner's 5 inner
     center rows per point: Ws[p] = [bias_kj | 2*IC_kj]*5 | k*.
  4. DVE: U = Ws * [1|x2h] broadcast (fp16 2x mode); segmented reduce-sum
     over d=33 -> fsel[p, 5] = the 5 inner scores of the winner only.
  5. j* via reduce-max + is_equal + iota dot (batched); id = 5k* + j*.
Data-parallel across 8 cores: x sharded along N (p-major contiguous DMA),
codebooks replicated. DMA ~68us/core; DVE is the modeled bottleneck.
"""

import os

import numpy as np

import bass_rust
import concourse.bass as bass
import concourse.mybir as mybir
import concourse.tile as tile
from concourse.bass_utils import run_bass_kernel_spmd
from concourse.masks import make_identity

N_TOTAL = 1_000_000
D = 48
D1 = 16
D2 = 32
KO = 64          # outer clusters
KI = 5           # inner per outer
DD = 33          # [bias-slot=1] + 32 x2 dims per inner center row
NW = KI * DD + 1  # 166: 5 blocks of 33 + k column
K1 = 51          # outer matmul contraction: 16 xh1 + 16 xl1 + 16 xh1 + 3 bias

N_CORES = 8
SUPER = 4096                          # points per DMA
CPS = SUPER // 128                    # 32 chunks per super
SB1 = 8                               # stage-1 batch (chunks)
SB2 = 8                               # stage-2 batch (chunks)
SB1_H = 4                             # xt-copy split point (ACT | DVE)

LAST_RESULTS = None

fp32 = mybir.dt.float32
fp16 = mybir.dt.float16


def _pad_to(n, m):
    return ((n + m - 1) // m) * m


def build_weights(outer_centers: np.ndarray, inner_centers: np.ndarray):
    """W1 [51, 64] fp16 outer weights; Wf [64, 166] fp16 inner gather table."""
    oc = outer_centers.astype(np.float64)
    ic = inner_centers.astype(np.float64)

    W1 = np.zeros((K1, KO), dtype=np.float32)
    och = (2.0 * oc.T).astype(np.float16).astype(np.float32)
    ocl = (2.0 * oc.T - och).astype(np.float16).astype(np.float32)
    W1[0:16] = och
    W1[16:32] = och
    W1[32:48] = ocl
    bias = -np.sum(oc * oc, axis=1)
    b0 = bias.astype(np.float16).astype(np.float64)
    r = bias - b0
    b1 = r.astype(np.float16).astype(np.float64)
    b2 = (r - b1).astype(np.float16)
    W1[48] = b0
    W1[49] = b1
    W1[50] = b2

    Wf = np.zeros((KO, NW), dtype=np.float32)
    for j in range(KI):
        Wf[:, j * DD] = -np.sum(ic[:, j, :] ** 2, axis=1)
        Wf[:, j * DD + 1 : (j + 1) * DD] = 2.0 * ic[:, j, :]
    Wf[:, KI * DD] = np.arange(KO)
    return W1.astype(np.float16), Wf.astype(np.float16)


def split_waits(nc):
    """Split multi-wait instructions into same-engine 1-wait NoOps + inst."""
    for f in nc.m.functions:
        for b in f.blocks:
            out = []
            for inst in b.instructions:
                si = inst.sync_info
                if si is not None and len(si.on_wait) > 1:
                    waits = list(si.on_wait)
                    for i, w in enumerate(waits[:-1]):
                        nop = mybir.InstNoOp(name=f"{inst.name}-sw{i}", ins=[], outs=[])
                        nop.engine = inst.engine
                        nop.sync_info = bass_rust.SyncInfo(on_wait=[w], on_update=[])
                        out.append(nop)
                    inst.sync_info = bass_rust.SyncInfo(
                        on_wait=[waits[-1]], on_update=list(si.on_update)
                    )
                out.append(inst)
            b.instructions = out


def build_program(n_pad: int, for_hw: bool = True):
    assert n_pad % SUPER == 0
    n_super = n_pad // SUPER

    nc = bass.Bass()
    x_ext = nc.declare_dram_parameter("x", [n_pad, D], fp32, isOutput=False)
    w1_ext = nc.declare_dram_parameter("w1", [K1, KO], fp16, isOutput=False)
    wf_ext = nc.declare_dram_parameter("wf", [KO, NW], fp16, isOutput=False)
    out_ext = nc.declare_dram_parameter("out", [n_pad], mybir.dt.int32, isOutput=True)

    with tile.TileContext(nc) as tc:
        with (
            tc.tile_pool(name="const", bufs=1) as constp,
            tc.tile_pool(name="xin", bufs=3) as xinp,
            tc.tile_pool(name="xo", bufs=3) as xop,
            tc.tile_pool(name="xi", bufs=3) as xip,
            tc.tile_pool(name="xt", bufs=1) as xtp,
            tc.tile_pool(name="m8", bufs=4) as m8p,
            tc.tile_pool(name="hh", bufs=4) as hhp,
            tc.tile_pool(name="ht", bufs=3) as htp,
            tc.tile_pool(name="wsel", bufs=3) as wsp,
            tc.tile_pool(name="uu", bufs=3) as uup,
            tc.tile_pool(name="fsel", bufs=3) as fselp,
            tc.tile_pool(name="jj", bufs=3) as jjp,
            tc.tile_pool(name="ids", bufs=2) as idsp,
            tc.tile_pool(name="psO", bufs=1, space="PSUM") as psOp,
            tc.tile_pool(name="pstO", bufs=2, space="PSUM") as pstOp,
            tc.tile_pool(name="pstH", bufs=1, space="PSUM") as pstHp,
            tc.tile_pool(name="pW", bufs=1, space="PSUM") as pWp,
        ):
            # ---- constants ----
            identF = constp.tile([128, 128], fp16)
            make_identity(nc, identF[:])

            jio_i = constp.tile([128, KI], mybir.dt.int32)
            nc.gpsimd.iota(jio_i[:], pattern=[[1, KI]], base=0, channel_multiplier=0)
            jiota = constp.tile([128, KI], fp16)
            nc.gpsimd.tensor_copy(jiota[:], jio_i[:])

            w1_sb = constp.tile([K1, KO], fp16)
            nc.sync.dma_start(out=w1_sb[:], in_=w1_ext[:])
            wf_sb = constp.tile([KO, NW], fp16)
            nc.sync.dma_start(out=wf_sb[:], in_=wf_ext[:])

            # persistent xt slots with preset ones rows (48:51)
            xt_slots = [
                xtp.tile([K1, SB1, 128], fp16, tag=f"xt{i}", name=f"xt{i}")
                for i in range(2)
            ]
            for t_ in xt_slots:
                nc.gpsimd.memset(t_[:], 1.0)

            # ---- main loop ----
            def do_super(s):
                xb = xinp.tile([128, CPS, D], fp32)
                src = x_ext[s * SUPER : (s + 1) * SUPER, :].rearrange(
                    "(p a) d -> p a d", p=128
                )
                nc.sync.dma_start(out=xb[:], in_=src)

                xo = xop.tile([128, CPS, D], fp16)
                nc.gpsimd.tensor_copy(xo[:, :, 0:16], xb[:, :, 0:16])
                nc.gpsimd.tensor_tensor(
                    out=xo[:, :, 16:32],
                    in0=xb[:, :, 0:16],
                    in1=xo[:, :, 0:16],
                    op=mybir.AluOpType.subtract,
                )
                nc.gpsimd.tensor_copy(xo[:, :, 32:48], xo[:, :, 0:16])
                xi = xip.tile([128, CPS, DD], fp16)
                nc.gpsimd.memset(xi[:, :, 0:1], 1.0)
                nc.gpsimd.tensor_copy(xi[:, :, 1:DD], xb[:, :, 16:48])

                ids_f = idsp.tile([128, CPS], fp16, tag="idsf")

                for g1 in range(CPS // SB1):
                    c0 = g1 * SB1
                    pstO = pstOp.tile([D, SB1, 128], fp16)
                    for b in range(SB1):
                        nc.tensor.transpose(
                            pstO[:, b, :], xo[:, c0 + b, :], identF[:]
                        )
                    xt = xt_slots[g1 % 2]
                    nc.scalar.copy(xt[0:D, :, :], pstO[:])

                    psO = psOp.tile([128, SB1, KO], fp32)
                    for b in range(SB1):
                        nc.tensor.matmul(
                            psO[:, b, :],
                            lhsT=xt[:, b, :],
                            rhs=w1_sb[:],
                            start=True,
                            stop=True,
                        )

                    m8 = m8p.tile([128, SB1], fp32)
                    nc.vector.tensor_reduce(
                        out=m8[:],
                        in_=psO[:],
                        axis=mybir.AxisListType.X,
                        op=mybir.AluOpType.max,
                    )
                    hh = hhp.tile([128, SB1, KO], fp16)
                    nc.vector.tensor_tensor(
                        out=hh[:],
                        in0=psO[:],
                        in1=m8[:].unsqueeze(-1).broadcast_to([128, SB1, KO]),
                        op=mybir.AluOpType.is_equal,
                    )

                    fsel = fselp.tile([128, SB1, KI], fp32)
                    kacc = jjp.tile([128, SB1], fp16, tag="kacc")
                    for g2 in range(SB1 // SB2):
                        b0 = g2 * SB2
                        pstH = pstHp.tile([KO, SB2, 128], fp16)
                        for b in range(SB2):
                            nc.tensor.transpose(
                                pstH[:, b, :], hh[:, b0 + b, :], identF[:]
                            )
                        ht = htp.tile([KO, SB2, 128], fp16)
                        nc.scalar.copy(ht[:], pstH[:])

                        pW = pWp.tile([128, SB2, 256], fp32)
                        for b in range(SB2):
                            nc.tensor.matmul(
                                pW[:, b, 0:NW],
                                lhsT=ht[:, b, :],
                                rhs=wf_sb[:],
                                start=True,
                                stop=True,
                            )
                        ws = wsp.tile([128, SB2, NW], fp16)
                        nc.scalar.copy(ws[:], pW[:, :, 0:NW])

                        uu = uup.tile([128, SB2, KI, DD], fp16)
                        xiv = (
                            xi[:, c0 + b0 : c0 + b0 + SB2, :]
                            .unsqueeze(2)
                            .broadcast_to([128, SB2, KI, DD])
                        )
                        nc.gpsimd.tensor_tensor(
                            out=uu[:],
                            in0=ws[:, :, 0 : KI * DD].rearrange(
                                "p b (j d) -> p b j d", j=KI
                            ),
                            in1=xiv,
                            op=mybir.AluOpType.mult,
                        )
                        nc.vector.tensor_reduce(
                            out=fsel[:, b0 : b0 + SB2, :],
                            in_=uu[:],
                            axis=mybir.AxisListType.X,
                            op=mybir.AluOpType.add,
                        )
                        nc.vector.tensor_copy(
                            kacc[:, b0 : b0 + SB2], ws[:, :, KI * DD]
                        )

                    # j* block (batched over SB1 chunks)
                    mf = jjp.tile([128, SB1], fp32, tag="mf")
                    nc.vector.tensor_reduce(
                        out=mf[:],
                        in_=fsel[:],
                        axis=mybir.AxisListType.X,
                        op=mybir.AluOpType.max,
                    )
                    ee = jjp.tile([128, SB1, KI], fp16, tag="ee")
                    nc.vector.tensor_tensor(
                        out=ee[:],
                        in0=fsel[:],
                        in1=mf[:].unsqueeze(-1).broadcast_to([128, SB1, KI]),
                        op=mybir.AluOpType.is_equal,
                    )
                    e2 = jjp.tile([128, SB1, KI], fp16, tag="e2")
                    nc.gpsimd.tensor_tensor(
                        out=e2[:],
                        in0=ee[:],
                        in1=jiota[:].unsqueeze(1).broadcast_to([128, SB1, KI]),
                        op=mybir.AluOpType.mult,
                    )
                    jf = jjp.tile([128, SB1], fp32, tag="jf")
                    nc.vector.tensor_reduce(
                        out=jf[:],
                        in_=e2[:],
                        axis=mybir.AxisListType.X,
                        op=mybir.AluOpType.add,
                    )
                    nc.vector.scalar_tensor_tensor(
                        out=ids_f[:, c0 : c0 + SB1],
                        in0=kacc[:],
                        scalar=5.0,
                        in1=jf[:],
                        op0=mybir.AluOpType.mult,
                        op1=mybir.AluOpType.add,
                    )

                ids_i = idsp.tile([128, CPS], mybir.dt.int32, tag="idsi")
                nc.vector.tensor_copy(ids_i[:], ids_f[:])
                dst = out_ext[s * SUPER : (s + 1) * SUPER].rearrange(
                    "(p a) -> p a", p=128
                )
                nc.sync.dma_start(out=dst, in_=ids_i[:])

            for s in range(n_super):
                do_super(s)
    if for_hw:
        split_waits(nc)
    return nc


def kernel(x, outer_centers, inner_centers):
    global LAST_RESULTS
    x = np.ascontiguousarray(np.asarray(x, dtype=np.float32))
    W1, Wf = build_weights(np.asarray(outer_centers), np.asarray(inner_centers))

    n = x.shape[0]
    shard = (n + N_CORES - 1) // N_CORES
    n_pad = _pad_to(shard, SUPER)

    nc = build_program(n_pad)

    in_maps = []
    for i in range(N_CORES):
        xs = x[i * shard : min((i + 1) * shard, n)]
        if xs.shape[0] < n_pad:
            xs = np.pad(xs, ((0, n_pad - xs.shape[0]), (0, 0)))
        in_maps.append({"x": xs, "w1": W1, "wf": Wf})

    res = run_bass_kernel_spmd(
        nc,
        in_maps,
        list(range(N_CORES)),
        trace=False,
    )
    LAST_RESULTS = res
    outs = []
    for i in range(N_CORES):
        lo = i * shard
        hi = min((i + 1) * shard, n)
        outs.append(res.results[i]["out"][: hi - lo])
    return np.concatenate(outs).astype(np.int32)

